# revision 44
# baseline (speedup 1.0000x reference)
"""Trainium2 Bass kernel for nn_MiniAttentionBlock.

Reference computation (B=16, S=4096, F=512):
    h      = tanh(x @ W + b)        [B,S,F]
    scores = h @ u                  [B,S]
    e      = exp(scores)
    a      = e / (sum(e) + eps)     global normalization over all B*S
    out    = sum_s x[b,s,:] * a[b,s]  -> [B,F]

Strategy: data-parallel over batch, 2 batches per core (8 cores).  The
global denominator is one tiny AllReduce of the per-core sum (shipped
replicated on 128 partitions so no broadcast is needed afterwards).
Each core receives its x shard pre-transposed and pre-tiled to
[NSB, 128, NKC, SB] fp16 so every superblock load is one fully
contiguous 1 MB DMA (8 KB per partition line).  x is shipped exactly
ONCE (fp16), serving both the TensorE matmul path and the DVE pooling
path; fp16 keeps the score noise ~8x below bf16.

Per 1024-row superblock (fp16 matmuls, fp32 PSUM):
  - h^T[g, rows] accumulated over 4 k-chunks per g-chunk into a
    [128, 1024] 2-bank PSUM tile (1024-col moving streams amortize the
    per-matmul restart overhead)
  - tanh(+bias) per g-chunk on ScalarE, PSUM -> SBUF fp16
  - scores matmul uses u replicated 128x along the stationary free dim
    so the PSUM result [128, rows] carries scores broadcast to all
    partitions; exp on ScalarE gives e (fp32) with the superblock's
    partial sum accumulated for free (accum_out)
  - weighted pooling sum_rows e*x runs on the DVE as a fused
    multiply + row-reduce custom op
The pooling of the last DEFER superblocks overlaps the AllReduce.
"""

import sys

if "/opt/trn_rl_repo" not in sys.path:
    sys.path.insert(0, "/opt/trn_rl_repo")

import numpy as np

from concourse import bass, bacc, tile, bass_utils
from concourse.dve_ops import TENSOR_TENSOR_REDUCE

mybir = bass.mybir

B, S, F = 16, 4096, 512
N_CORES = 8
BPC = B // N_CORES          # batches per core
R = BPC * S                 # rows per core
SB = 1024                   # rows per superblock
NSB = R // SB               # superblocks per core
DEFER = 3                   # trailing superblocks pooled during the AllReduce
NKC = F // 128              # 128-partition chunks of F
EPS = 1e-7

F32 = mybir.dt.float32
F16 = mybir.dt.float16
ALU = mybir.AluOpType
ACTF = mybir.ActivationFunctionType
AXIS = mybir.AxisListType

_CACHE = {}
_EYE = np.eye(128, dtype=np.float32)




def _build():
    nc = bacc.Bacc("TRN2", target_bir_lowering=False, debug=False,
                   num_devices=N_CORES)


    xh = nc.dram_tensor("xh", [NSB, 128, NKC, SB], F16, kind="ExternalInput")
    w = nc.dram_tensor("w", [F, F], F16, kind="ExternalInput")
    b2 = nc.dram_tensor("b2", [128, NKC], F32, kind="ExternalInput")
    ur = nc.dram_tensor("ur", [128, NKC, 128], F16, kind="ExternalInput")
    eye = nc.dram_tensor("eye", [128, 128], F32, kind="ExternalInput")
    # PE-transposed [BPC*NKC, 128] layout: the final DMA is 8 contiguous
    # 512 B rows instead of 128 32 B partition lines
    out = nc.dram_tensor("out", [BPC * NKC, 128], F32, kind="ExternalOutput")

    with tile.TileContext(nc) as tc:
        with tc.tile_pool(name="const", bufs=1) as cpool, \
             tc.tile_pool(name="xbp", bufs=DEFER + 4) as xbp, \
             tc.tile_pool(name="hap", bufs=8) as hap, \
             tc.tile_pool(name="erp", bufs=DEFER + 2) as erp, \
             tc.tile_pool(name="scr", bufs=4) as scr, \
             tc.tile_pool(name="hps", bufs=2, space="PSUM") as hps, \
             tc.tile_pool(name="sps", bufs=2, space="PSUM") as sps, \
             tc.tile_pool(name="dram", bufs=1, space="DRAM") as dram:

            # ---- PE warm-up: ~3.4us of dummy matmuls flips the HAM clock
            # gate to 8/8 (2.4 GHz) before the first real matmul arrives.
            # Zeros in, scratch PSUM out (never read) -> numerically inert.
            wsrc = cpool.tile([128, F], F16, tag="wsrc")
            nc.vector.memset(wsrc[:, 0:128], 0.0)
            wps = sps.tile([128, SB], F32, tag="s", name="wps")
            for _ in range(44):
                nc.tensor.matmul(wps[:, 0:128], lhsT=wsrc[:, 0:128],
                                 rhs=wsrc[:, 0:128], start=True, stop=True)



            # ---- constants: the transfers that gate the first matmul
            # group (4 W tiles + 4 x chunks of sb0/half0) are interleaved
            # on the two HWDGE queues, most-urgent first (the first
            # accumulation group consumes kc in order).
            h0cs = slice(0, SB // 2)
            w_sb = [cpool.tile([128, F], F16, tag=f"w{kc}", name=f"w{kc}")
                    for kc in range(NKC)]
            x0 = xbp.tile([128, NKC, SB], F16, tag="xb", name="x0")
            nc.scalar.dma_start(out=w_sb[0][:], in_=w.ap()[0:128, :])
            nc.sync.dma_start(out=x0[:, 0, h0cs], in_=xh.ap()[0, :, 0, h0cs])
            nc.scalar.dma_start(out=x0[:, 1, h0cs], in_=xh.ap()[0, :, 1, h0cs])
            nc.sync.dma_start(out=w_sb[1][:], in_=w.ap()[128:256, :])
            nc.scalar.dma_start(out=w_sb[2][:], in_=w.ap()[256:384, :])
            nc.sync.dma_start(out=x0[:, 2, h0cs], in_=xh.ap()[0, :, 2, h0cs])
            nc.scalar.dma_start(out=w_sb[3][:], in_=w.ap()[384:512, :])
            nc.sync.dma_start(out=x0[:, 3, h0cs], in_=xh.ap()[0, :, 3, h0cs])
            b_sb = cpool.tile([128, NKC], F32, tag="b")
            nc.scalar.dma_start(out=b_sb[:], in_=b2.ap())
            u_sb = cpool.tile([128, NKC, 128], F16, tag="u")
            nc.scalar.dma_start(out=u_sb[:], in_=ur.ap())
            eye_sb = cpool.tile([128, 128], F32, tag="eye")
            nc.scalar.dma_start(out=eye_sb[:], in_=eye.ap())

            # warmup collective: pre-warms the ncfw/credit machinery while
            # compute runs; its result is unused
            wu_in = dram.tile([1, 1], F32)
            wu_out = dram.tile([8, 1], F32, addr_space="Shared")
            wu_sb = cpool.tile([1, 1], F32, tag="wusb")
            nc.vector.memset(wu_sb[:], 0.0)
            nc.scalar.dma_start(out=wu_in[:], in_=wu_sb[:])
            nc.gpsimd.collective_compute(
                "AllGather", ALU.bypass,
                replica_groups=[list(range(N_CORES))],
                ins=[wu_in.opt()], outs=[wu_out.opt()])

            # column layout: sb0..6 full superblocks -> cols 0..6; the last
            # superblock is half-pipelined -> cols 7 (h0) and 8 (h1)
            NCOL = NSB + 1
            esum = cpool.tile([128, NCOL], F32, tag="esum")
            nums = [cpool.tile([128, NCOL], F32, tag=f"num{kc}", name=f"num{kc}")
                    for kc in range(NKC)]
            out_sb = cpool.tile([128, BPC, NKC], F32, tag="osb")

            # ---- main loop over superblocks ----
            hacts = {}          # sb -> [ha per mc]
            xtiles = {}         # sb -> x sbuf tile
            spsum = {}          # sb -> scores psum tile
            ers = {}            # sb -> exp sbuf tile

            def emit_scores(sb):
                sp = sps.tile([128, SB], F32, tag="s", name="sp")
                has = hacts.pop(sb)
                for half in range(2):
                    cs = slice(half * (SB // 2), (half + 1) * (SB // 2))
                    for mc in range(NKC):
                        nc.tensor.matmul(
                            sp[:, cs],
                            lhsT=u_sb[:, mc, :],
                            rhs=has[mc][:, cs],
                            start=(mc == 0), stop=(mc == NKC - 1))
                spsum[sb] = sp

            def emit_exp(sb):
                sp = spsum.pop(sb)
                er = erp.tile([128, SB], F32, tag="er", name="er")
                nc.scalar.activation(out=er[:], in_=sp[:], func=ACTF.Exp,
                                     accum_out=esum[:, sb:sb + 1])
                ers[sb] = er

            def emit_pool(sb):
                er = ers.pop(sb)
                xall = xtiles.pop(sb)
                for kc in range(NKC):
                    sc = scr.tile([128, SB], F16, tag="sc", name="sc")
                    nc.vector._custom_dve(
                        TENSOR_TENSOR_REDUCE,
                        out=sc[:], in0=xall[:, kc, :], in1=er[:],
                        s0=0.0, s1=1.0,
                        accum_out=nums[kc][:, sb:sb + 1])

            def emit_h_group(hps_t, xall, mc):
                for half in range(2):
                    cs = slice(half * (SB // 2), (half + 1) * (SB // 2))
                    for kc in range(NKC):
                        nc.tensor.matmul(
                            hps_t[mc][:, cs],
                            lhsT=w_sb[kc][:, mc * 128:(mc + 1) * 128],
                            rhs=xall[:, kc, cs],
                            start=(kc == 0), stop=(kc == NKC - 1))

            for sb in range(NSB):
                if sb == 0:
                    xall = x0
                    # half 0 was issued up top; issue half 1 now
                    cs = slice(SB // 2, SB)
                    for kc in range(NKC):
                        nc.sync.dma_start(
                            out=xall[:, kc, cs],
                            in_=xh.ap()[sb, :, kc, cs])
                else:
                    xall = xbp.tile([128, NKC, SB], F16, tag="xb", name="xall")
                    nc.sync.dma_start(out=xall[:], in_=xh.ap()[sb])
                xtiles[sb] = xall

                # h^T[g, rows] = sum_f W[f, g] * xT[f, rows]
                # (512-col matmuls: a PSUM bank holds 512 fp32, so each
                # [128, 1024] 2-bank tile is filled as two half sweeps)
                hps_t = [hps.tile([128, SB], F32, tag="h", name="hp")
                         for _ in range(NKC)]
                if sb == 0:
                    # half-major: the first four chunk DMAs feed a full
                    # half-sweep over all mc, halving the startup stall
                    for mc, half in [(mc, half) for half in range(2)
                                     for mc in range(NKC)]:
                        cs = slice(half * (SB // 2), (half + 1) * (SB // 2))
                        for kc in range(NKC):
                            nc.tensor.matmul(
                                hps_t[mc][:, cs],
                                lhsT=w_sb[kc][:, mc * 128:(mc + 1) * 128],
                                rhs=xall[:, kc, cs],
                                start=(kc == 0), stop=(kc == NKC - 1))
                else:
                    # mc0 first; scores(sb-1) slots in behind it so exp and
                    # pooling of sb-1 complete within this iteration
                    emit_h_group(hps_t, xall, 0)
                    emit_scores(sb - 1)
                    emit_exp(sb - 1)
                    for mc in range(1, NKC):
                        emit_h_group(hps_t, xall, mc)

                # tanh(+bias), PSUM -> SBUF fp16
                has = []
                for mc in range(NKC):
                    ha = hap.tile([128, SB], F16, tag="h", name="ha")
                    nc.scalar.activation(out=ha[:], in_=hps_t[mc][:],
                                         func=ACTF.Tanh,
                                         bias=b_sb[:, mc:mc + 1])
                    has.append(ha)
                hacts[sb] = has

                if sb >= 1:
                    emit_pool(sb - 1)

            # ---- last superblock: half-pipelined scores -> exp so the
            # collective triggers as early as possible
            sp7 = sps.tile([128, SB], F32, tag="s", name="sp7")
            has7 = hacts.pop(NSB - 1)
            x7 = xtiles.pop(NSB - 1)
            er7 = []
            for half in range(2):
                cs = slice(half * (SB // 2), (half + 1) * (SB // 2))
                for mc in range(NKC):
                    nc.tensor.matmul(
                        sp7[:, cs],
                        lhsT=u_sb[:, mc, :],
                        rhs=has7[mc][:, cs],
                        start=(mc == 0), stop=(mc == NKC - 1))
                er_h = erp.tile([128, SB // 2], F32, tag="er", name="er7")
                nc.scalar.activation(out=er_h[:], in_=sp7[:, cs],
                                     func=ACTF.Exp,
                                     accum_out=esum[:, NSB - 1 + half:NSB + half])
                er7.append(er_h)

            # ---- gather the raw per-superblock partials of all cores (the
            # sum over all 72 values happens after the gather): skips one
            # reduce hop on the pre-collective critical path
            cc_in = dram.tile([1, NCOL], F32)
            cc_out = dram.tile([8, NCOL], F32, addr_space="Shared")
            nc.sync.dma_start(out=cc_in[:], in_=esum[0:1, :])
            nc.gpsimd.collective_compute(
                "AllGather", ALU.bypass,
                replica_groups=[list(range(N_CORES))],
                ins=[cc_in.opt()], outs=[cc_out.opt()])

            # pooling for the last superblock, overlapping the exchange prep
            for half in range(2):
                cs = slice(half * (SB // 2), (half + 1) * (SB // 2))
                for kc in range(NKC):
                    sc = scr.tile([128, SB // 2], F16, tag="sc", name="sc7")
                    nc.vector._custom_dve(
                        TENSOR_TENSOR_REDUCE,
                        out=sc[:], in0=x7[:, kc, cs], in1=er7[half][:],
                        s0=0.0, s1=1.0,
                        accum_out=nums[kc][:, NSB - 1 + half:NSB + half])

            # per-batch reduction, PE-transpose to 8 partitions, PSUM->SBUF
            for kc in range(NKC):
                nc.vector.tensor_reduce(
                    out=out_sb[:, 0, kc:kc + 1],
                    in_=nums[kc][:, 0:NSB // BPC],
                    axis=AXIS.X, op=ALU.add)
                nc.vector.tensor_reduce(
                    out=out_sb[:, 1, kc:kc + 1],
                    in_=nums[kc][:, NSB // BPC:NCOL],
                    axis=AXIS.X, op=ALU.add)
            pt = sps.tile([128, 128], F32, tag="s", name="pt")
            nc.tensor.transpose(
                pt[0:BPC * NKC, :],
                out_sb[:].rearrange("p b c -> p (b c)"),
                eye_sb[:])
            pts = cpool.tile([BPC * NKC, 128], F32, tag="pts")
            nc.scalar.copy(out=pts[:], in_=pt[0:BPC * NKC, :])

            # post-collective critical path: one DMA lands all 72 gathered
            # partials replicated on 8 partitions (0-stride src), then the
            # whole normalize chain stays on the Vector queue. (EPS=1e-7 on
            # a ~1e5 denominator is below fp32 resolution -> omitted.)
            sg8r = cpool.tile([BPC * NKC, 8 * NCOL], F32, tag="sg8")
            nc.sync.dma_start(
                out=sg8r[:],
                in_=cc_out[:].rearrange("a b -> () (a b)")
                .partition_broadcast(BPC * NKC))
            rcp8 = cpool.tile([BPC * NKC, 1], F32, tag="rcp")
            nc.vector.tensor_reduce(out=rcp8[:], in_=sg8r[:],
                                    axis=AXIS.X, op=ALU.add)
            nc.vector.reciprocal(out=rcp8[:], in_=rcp8[:])
            nc.vector.tensor_scalar_mul(out=pts[:], in0=pts[:],
                                        scalar1=rcp8[:])
            nc.sync.dma_start(out=out.ap(), in_=pts[:])

    nc.compile()
    return nc


def _get_compiled():
    if "nc" not in _CACHE:
        _CACHE["nc"] = _build()
    return _CACHE["nc"]


def _make_in_maps(x, W, b, u):
    Wc = np.ascontiguousarray(np.asarray(W, np.float32).astype(np.float16))
    bc = np.ascontiguousarray(np.asarray(b, np.float32).reshape(NKC, 128).T)
    u_cols = np.asarray(u, np.float32).reshape(NKC, 128).T  # [128, NKC]
    urc = np.ascontiguousarray(
        np.broadcast_to(u_cols[:, :, None], (128, NKC, 128))
    ).astype(np.float16)
    in_maps = []
    for c in range(N_CORES):
        xc = np.asarray(x[BPC * c:BPC * (c + 1)], np.float32).reshape(R, F)
        xt = np.ascontiguousarray(
            xc.T.reshape(NKC, 128, NSB, SB).transpose(2, 1, 0, 3)
        ).astype(np.float16)
        in_maps.append({"xh": xt, "w": Wc, "b2": bc, "ur": urc,
                        "eye": _EYE})
    return in_maps


def kernel(x, W, b, u):
    nc = _get_compiled()
    in_maps = _make_in_maps(x, W, b, u)
    res = bass_utils.run_bass_kernel_spmd(
        nc, in_maps, core_ids=list(range(N_CORES)))
    _CACHE["last_results"] = res
    return np.concatenate(
        [res.results[c]["out"].reshape(BPC, F) for c in range(N_CORES)],
        axis=0)


def kernel_traced(x, W, b, u, **trace_kwargs):
    """Same as kernel() but with NTFF tracing; returns (out, BassKernelResults)."""
    nc = _get_compiled()
    in_maps = _make_in_maps(x, W, b, u)
    res = bass_utils.run_bass_kernel_spmd(
        nc, in_maps, core_ids=list(range(N_CORES)), trace=True, **trace_kwargs)
    _CACHE["last_results"] = res
    out = np.concatenate(
        [res.results[c]["out"].reshape(BPC, F) for c in range(N_CORES)],
        axis=0)
    return out, res

